# revision 27
# baseline (speedup 1.0000x reference)
import json
import sys

sys.path.insert(0, "/opt/trn_rl_repo")

import numpy as np

import concourse.bass_utils as _bu
import concourse.bass2jax as _b2j
import concourse.bass as bass
import concourse.mybir as mybir
from concourse import tile

# ---------------------------------------------------------------------------
# The walrus build in this container only supports ONE sync-wait per
# instruction; current Tile emits multi-wait instructions. Split the extra
# waits into single-wait NoOps on the same engine (engines execute their
# stream in order, so semantics are identical).
_orig_compile_bir = _bu.compile_bir_kernel


def _split_multiwaits(bir_bytes):
    d = json.loads(bir_bytes)
    n = 0
    for fn in d["functions"]:
        for blk in fn["blocks"]:
            out = []
            for ins in blk["instructions"]:
                si = ins.get("sync_info")
                waits = (si or {}).get("on_wait") or []
                if len(waits) > 1:
                    for w in waits[:-1]:
                        n += 1
                        out.append(
                            {
                                "name": f"WSPL{n}-{ins['name']}",
                                "opcode": "NoOp",
                                "engine": ins["engine"],
                                "debug": ins.get("debug", 0),
                                "ins": [],
                                "outs": [],
                                "sync_info": {"on_wait": [w]},
                            }
                        )
                    si["on_wait"] = [waits[-1]]
                out.append(ins)
            blk["instructions"] = out
    return json.dumps(d).encode()


def _patched_compile_bir(bir_json, tmpdir, neff_name="file.neff"):
    return _orig_compile_bir(_split_multiwaits(bir_json), tmpdir, neff_name)


if getattr(_bu.compile_bir_kernel, "__name__", "") != "_patched_compile_bir":
    _bu.compile_bir_kernel = _patched_compile_bir
    _b2j.compile_bir_kernel = _patched_compile_bir
# ---------------------------------------------------------------------------

# nn_MinConv2dGRUCell: x (4,32,64,32,32), h0 (4,1,64,32,32), W (128,64,3,3),
# b (128,). out = (4,32,64,32,32), h_next = out[:, -1:].
# Sharding: 8 cores = B(4) x H-half(2). Per core: conv over its 16 rows
# (+1 halo row each side), then the minGRU scan h_s = a_s*h_{s-1} + bv_s with
# a = sigmoid(-gate-b_g), bv = (1-a)*g, g = max(hidden+b_h+0.5, sigmoid(hidden+b_h)).
B, S, C_IN, H, WD = 4, 32, 64, 32, 32
HID = 64
N_CORES = 8
HH = H // 2  # 16 rows per core
PX = HH * WD  # 512 pixels per core
HPX = PX // 2  # 256 = free size after (128,256) repack
BLK = 8  # timesteps per tensor_tensor_scan instruction
NBLK = S // BLK
SEG = BLK + 1  # +1 reset column per pixel-tuple

F32 = mybir.dt.float32
F32R = mybir.dt.float32r
BF16 = mybir.dt.bfloat16

_CACHE = {}


def _build_nc():
    nc = bass.Bass(trn_type="TRN2")

    xpad_d = nc.dram_tensor("xpad", (S, C_IN, 18, 34), BF16, kind="ExternalInput")
    # paired taps (ky,0)+(ky,1) stacked on K; leftover taps (ky,2) separate
    wtp_d = nc.dram_tensor("wtp", (2 * C_IN, 3, 2 * HID), BF16, kind="ExternalInput")
    wtq_d = nc.dram_tensor("wtq", (2 * C_IN, 2 * HID), BF16, kind="ExternalInput")
    wts_d = nc.dram_tensor("wts", (C_IN, 2 * HID), BF16, kind="ExternalInput")
    h0_d = nc.dram_tensor("h0r", (HID, PX), F32, kind="ExternalInput")
    # bias columns: 0 = -b_gate (rows 0:64), 1 = b_hid, 2 = b_hid+0.5
    # (rows 64:128), 3 = +b_gate (rows 0:64)
    bias_d = nc.dram_tensor("biases", (128, 4), F32, kind="ExternalInput")
    # raw scan-layout output: [block, channel, px*SEG] — host strips reset
    # columns and reorders; keeps the device-side DMA fully contiguous.
    out_d = nc.dram_tensor("out", (NBLK, HID, PX * SEG), F32, kind="ExternalOutput")

    AL = mybir.AluOpType
    ACTF = mybir.ActivationFunctionType

    with tile.TileContext(nc) as tc:
        with (
            tc.tile_pool(name="const", bufs=1) as constp,
            tc.tile_pool(name="xin", bufs=4) as xinp,
            tc.tile_pool(name="psum", bufs=8, space="PSUM") as psump,
            tc.tile_pool(name="ew", bufs=4) as ewp,
            tc.tile_pool(name="scan", bufs=2) as scanp,
            tc.tile_pool(name="outb", bufs=2) as outp,
        ):
            wtp_t = constp.tile([2 * C_IN, 3 * 2 * HID], BF16)
            nc.sync.dma_start(wtp_t[:], wtp_d[:, :, :])
            wtq_t = constp.tile([2 * C_IN, 2 * HID], BF16)
            nc.sync.dma_start(wtq_t[:], wtq_d[:, :])
            wts_t = constp.tile([C_IN, 2 * HID], BF16)
            nc.sync.dma_start(wts_t[:], wts_d[:, :])
            bias_t = constp.tile([128, 4], F32)
            nc.sync.dma_start(bias_t[:], bias_d[:, :])
            h0_t = constp.tile([HID, PX], F32)
            nc.sync.dma_start(h0_t[:], h0_d[:, :])

            o_prev = None
            for bs in range(NBLK):
                a_t = scanp.tile([HID, PX * SEG], F32, tag="a")
                bv_t = scanp.tile([HID, PX * SEG], F32, tag="bv")
                a3 = a_t[:].rearrange("p (px j) -> p px j", j=SEG)
                bv3 = bv_t[:].rearrange("p (px j) -> p px j", j=SEG)

                # reset columns: a=0, bv=h_init  =>  state restarts at h_init
                nc.gpsimd.memset(a3[:, :, 0], 0.0)
                if bs == 0:
                    nc.vector.tensor_copy(bv3[:, :, 0], h0_t[:])
                else:
                    op3 = o_prev[:].rearrange("p (px j) -> p px j", j=SEG)
                    nc.vector.tensor_copy(bv3[:, :, 0], op3[:, :, SEG - 1])

                for j in range(BLK):
                    s = bs * BLK + j
                    # x tile: partitions 0:64 = xpad, 64:128 = xpad shifted
                    # left by one column (tap kx+1 when read at kx)
                    x_t = xinp.tile([2 * C_IN, 2 * 18 * 34], BF16)
                    x3 = x_t[:].rearrange("p (q r c) -> p q r c", q=2, r=18)
                    xflat = xpad_d[s, :, :, :].rearrange("c r w -> c (r w)")
                    nc.sync.dma_start(x3[0:C_IN, 0, :, :], xpad_d[s, :, :, :])
                    # bottom A: xpad shifted left 1 col (tap kx+1 at kx);
                    # row-boundary bleed lands in pad col 33 (never read)
                    nc.sync.dma_start(x_t[C_IN:, 0:611], xflat[:, 1:612])
                    nc.sync.dma_start(x3[0:C_IN, 1, :, :], xpad_d[s, :, :, :])
                    # bottom B: xpad shifted up one row (tap ky+1 at ky)
                    nc.sync.dma_start(x_t[C_IN:, 612:1190], xflat[:, 34:612])

                    p_t = psump.tile([128, PX], F32)
                    for g in range(3):
                        # taps (g,0)+(g,1) in one K=128 matmul
                        nc.tensor.matmul(
                            p_t[:],
                            wtp_t[:, g * 128 : (g + 1) * 128],
                            x3[:, 0, g : g + HH, 0:WD],
                            start=(g == 0),
                            stop=False,
                        )
                    # taps (0,2)+(1,2) in one K=128 matmul via row-shifted copy
                    nc.tensor.matmul(
                        p_t[:],
                        wtq_t[:],
                        x3[:, 1, 0:HH, 2 : 2 + WD],
                        start=False,
                        stop=False,
                    )
                    # leftover tap (2,2), K=64
                    nc.tensor.matmul(
                        p_t[:],
                        wts_t[:],
                        x3[0:C_IN, 0, 2 : 2 + HH, 2 : 2 + WD],
                        start=False,
                        stop=True,
                    )

                    # a = sigmoid(-gate - b_g)   [partitions 0:64]
                    nc.scalar.activation(
                        a3[:, :, 1 + j],
                        p_t[0:HID, :],
                        ACTF.Sigmoid,
                        bias=bias_t[0:HID, 0:1],
                        scale=-1.0,
                    )
                    # [z; sig_h] = sigmoid(psum + [b_g; b_h]) in one op
                    sh_t = ewp.tile([128, PX], BF16, tag="sh")
                    nc.scalar.activation(
                        sh_t[:], p_t[:], ACTF.Sigmoid, bias=bias_t[:, 1:2]
                    )
                    # g = max(hidden + b_h + 0.5, sig_h)   [partitions 64:128]
                    g_t = ewp.tile([128, PX], BF16, tag="g")
                    nc.vector.scalar_tensor_tensor(
                        g_t[HID:, :], p_t[HID:, :], bias_t[HID:, 2:3], sh_t[HID:, :],
                        op0=AL.add, op1=AL.max,
                    )
                    # shift g down to the gate lanes
                    gl_t = ewp.tile([HID, PX], BF16, tag="gl")
                    nc.sync.dma_start(gl_t[:], g_t[HID:, :])
                    # z = 1 - a: alternate engines to balance ACT vs DVE load
                    # bv = z * g   (Pool engine, frees DVE for the scan)
                    nc.gpsimd.tensor_tensor(
                        bv3[:, :, 1 + j], sh_t[0:HID, :], gl_t[:], op=AL.mult
                    )

                o_t = outp.tile([HID, PX * SEG], F32, tag="o")
                # h = a*h + bv along the segmented free dim
                nc.vector.tensor_tensor_scan(
                    o_t[:], a_t[:], bv_t[:], 0.0, op0=AL.mult, op1=AL.add
                )
                nc.sync.dma_start(out_d[bs, :, :], o_t[:])
                o_prev = o_t

    return nc


def kernel(x, h0, W, b):
    x = np.ascontiguousarray(x, dtype=np.float32)
    import ml_dtypes
    h0 = np.ascontiguousarray(h0, dtype=np.float32)
    W = np.ascontiguousarray(W, dtype=np.float32)
    b = np.ascontiguousarray(b, dtype=np.float32)

    if "nc" not in _CACHE:
        _CACHE["nc"] = _build_nc()
    nc = _CACHE["nc"]

    # host-side prep shared across cores
    wt = W.transpose(1, 2, 3, 0).reshape(C_IN, 9, 2 * HID)  # (ic, ky*3+kx, oc)
    wtp = np.ascontiguousarray(
        np.concatenate([wt[:, [0, 3, 6], :], wt[:, [1, 4, 7], :]], axis=0)
    ).astype(ml_dtypes.bfloat16)  # (128, 3, 128)
    wtq = np.ascontiguousarray(
        np.concatenate([wt[:, 2, :], wt[:, 5, :]], axis=0)
    ).astype(ml_dtypes.bfloat16)  # (128, 128)
    wts = np.ascontiguousarray(wt[:, 8, :]).astype(ml_dtypes.bfloat16)
    biases = np.zeros((128, 4), dtype=np.float32)
    biases[0:HID, 0] = -b[0:HID]
    biases[0:HID, 1] = b[0:HID]
    biases[HID:, 1] = b[HID:]
    biases[HID:, 2] = b[HID:] + 0.5

    in_maps = []
    for core in range(N_CORES):
        bi, hh = core // 2, core % 2
        xpad = np.zeros((S, C_IN, 18, 34), dtype=ml_dtypes.bfloat16)
        r0 = hh * HH - 1  # global row of xpad row 0
        lo, hi = max(r0, 0), min(r0 + 18, H)
        xpad[:, :, lo - r0 : hi - r0, 1:33] = x[bi, :, :, lo:hi, :]
        h0r = np.ascontiguousarray(
            h0[bi, 0, :, hh * HH : (hh + 1) * HH, :].reshape(HID, PX)
        )
        in_maps.append(
            {"xpad": xpad, "wtp": wtp, "wtq": wtq, "wts": wts, "h0r": h0r, "biases": biases}
        )

    res = _bu.run_bass_kernel_spmd(nc, in_maps, core_ids=list(range(N_CORES)))

    out = np.empty((B, S, HID, H, WD), dtype=np.float32)
    for core in range(N_CORES):
        bi, hh = core // 2, core % 2
        raw = res.results[core]["out"].reshape(NBLK, HID, PX, SEG)[:, :, :, 1:]
        # (blk, c, px, j) -> (s=blk*BLK+j, c, y, x)
        core_out = raw.transpose(0, 3, 1, 2).reshape(S, HID, HH, WD)
        out[bi, :, :, hh * HH : (hh + 1) * HH, :] = core_out
    h_next = out[:, -1:].copy()
    return out, h_next


# revision 28
# speedup vs baseline: 1.2255x; 1.2255x over previous
import json
import sys

sys.path.insert(0, "/opt/trn_rl_repo")

import numpy as np

import concourse.bass_utils as _bu
import concourse.bass2jax as _b2j
import concourse.bass as bass
import concourse.mybir as mybir
from concourse import tile

# ---------------------------------------------------------------------------
# The walrus build in this container only supports ONE sync-wait per
# instruction; current Tile emits multi-wait instructions. Split the extra
# waits into single-wait NoOps on the same engine (engines execute their
# stream in order, so semantics are identical).
_orig_compile_bir = _bu.compile_bir_kernel


def _split_multiwaits(bir_bytes):
    d = json.loads(bir_bytes)
    n = 0
    for fn in d["functions"]:
        for blk in fn["blocks"]:
            out = []
            for ins in blk["instructions"]:
                si = ins.get("sync_info")
                waits = (si or {}).get("on_wait") or []
                if len(waits) > 1:
                    for w in waits[:-1]:
                        n += 1
                        out.append(
                            {
                                "name": f"WSPL{n}-{ins['name']}",
                                "opcode": "NoOp",
                                "engine": ins["engine"],
                                "debug": ins.get("debug", 0),
                                "ins": [],
                                "outs": [],
                                "sync_info": {"on_wait": [w]},
                            }
                        )
                    si["on_wait"] = [waits[-1]]
                out.append(ins)
            blk["instructions"] = out
    return json.dumps(d).encode()


def _patched_compile_bir(bir_json, tmpdir, neff_name="file.neff"):
    return _orig_compile_bir(_split_multiwaits(bir_json), tmpdir, neff_name)


if getattr(_bu.compile_bir_kernel, "__name__", "") != "_patched_compile_bir":
    _bu.compile_bir_kernel = _patched_compile_bir
    _b2j.compile_bir_kernel = _patched_compile_bir
# ---------------------------------------------------------------------------

# nn_MinConv2dGRUCell: x (4,32,64,32,32), h0 (4,1,64,32,32), W (128,64,3,3),
# b (128,). out = (4,32,64,32,32), h_next = out[:, -1:].
# Sharding: 8 cores = B(4) x H-half(2). Per core: conv over its 16 rows
# (+1 halo row each side), then the minGRU scan h_s = a_s*h_{s-1} + bv_s with
# a = sigmoid(-gate-b_g), bv = (1-a)*g, g = max(hidden+b_h+0.5, sigmoid(hidden+b_h)).
B, S, C_IN, H, WD = 4, 32, 64, 32, 32
HID = 64
N_CORES = 8
HH = H // 2  # 16 rows per core
PX = HH * WD  # 512 pixels per core
HPX = PX // 2  # 256 = free size after (128,256) repack
BLK = 8  # timesteps per tensor_tensor_scan instruction
NBLK = S // BLK
SEG = BLK + 1  # +1 reset column per pixel-tuple

F32 = mybir.dt.float32
F32R = mybir.dt.float32r
BF16 = mybir.dt.bfloat16

_CACHE = {}


def _build_nc():
    nc = bass.Bass(trn_type="TRN2")

    xpad_d = nc.dram_tensor("xpad", (S, C_IN, 18, 34), BF16, kind="ExternalInput")
    # paired taps (ky,0)+(ky,1) stacked on K; leftover taps (ky,2) separate
    wtp_d = nc.dram_tensor("wtp", (2 * C_IN, 3, 2 * HID), BF16, kind="ExternalInput")
    wts_d = nc.dram_tensor("wts", (C_IN, 3, 2 * HID), BF16, kind="ExternalInput")
    h0_d = nc.dram_tensor("h0r", (HID, PX), F32, kind="ExternalInput")
    # bias columns: 0 = -b_gate (rows 0:64), 1 = b_hid, 2 = b_hid+0.5
    # (rows 64:128), 3 = +b_gate (rows 0:64)
    bias_d = nc.dram_tensor("biases", (128, 4), F32, kind="ExternalInput")
    # raw scan-layout output: [block, channel, px*SEG] — host strips reset
    # columns and reorders; keeps the device-side DMA fully contiguous.
    out_d = nc.dram_tensor("out", (S, HID, PX), F32, kind="ExternalOutput")

    AL = mybir.AluOpType
    ACTF = mybir.ActivationFunctionType

    with tile.TileContext(nc) as tc:
        with (
            tc.tile_pool(name="const", bufs=1) as constp,
            tc.tile_pool(name="xin", bufs=4) as xinp,
            tc.tile_pool(name="psum", bufs=8, space="PSUM") as psump,
            tc.tile_pool(name="ew", bufs=4) as ewp,
            tc.tile_pool(name="scan", bufs=2) as scanp,
            tc.tile_pool(name="outb", bufs=3) as outp,
        ):
            wtp_t = constp.tile([2 * C_IN, 3 * 2 * HID], BF16)
            nc.sync.dma_start(wtp_t[:], wtp_d[:, :, :])
            wts_t = constp.tile([C_IN, 3 * 2 * HID], BF16)
            nc.sync.dma_start(wts_t[:], wts_d[:, :, :])
            bias_t = constp.tile([128, 4], F32)
            nc.sync.dma_start(bias_t[:], bias_d[:, :])
            h0_t = constp.tile([HID, PX], F32)
            nc.sync.dma_start(h0_t[:], h0_d[:, :])

            hprev = h0_t
            for bs in range(NBLK):
                for j in range(BLK):
                    s = bs * BLK + j
                    # x tile: partitions 0:64 = xpad, 64:128 = xpad shifted
                    # left by one column (tap kx+1 when read at kx)
                    x_t = xinp.tile([2 * C_IN, 18 * 34], BF16)
                    x3 = x_t[:].rearrange("p (r c) -> p r c", r=18)
                    xflat = xpad_d[s, :, :, :].rearrange("c r w -> c (r w)")
                    nc.sync.dma_start(x3[0:C_IN, :, :], xpad_d[s, :, :, :])
                    # shifted copy: bottom[f] = xpad[f+1]; the row-boundary
                    # bleed lands in pad col 33, which no tap reads
                    nc.sync.dma_start(x_t[C_IN:, 0:611], xflat[:, 1:612])

                    p_t = psump.tile([128, PX], F32)
                    for g in range(3):
                        # taps (g,0)+(g,1) in one K=128 matmul
                        nc.tensor.matmul(
                            p_t[:],
                            wtp_t[:, g * 128 : (g + 1) * 128],
                            x3[:, g : g + HH, 0:WD],
                            start=(g == 0),
                            stop=False,
                        )
                    for g in range(3):
                        # leftover tap (g,2), K=64
                        nc.tensor.matmul(
                            p_t[:],
                            wts_t[:, g * 128 : (g + 1) * 128],
                            x3[0:C_IN, g : g + HH, 2 : 2 + WD],
                            start=False,
                            stop=(g == 2),
                        )

                    # a = sigmoid(-gate - b_g)   [partitions 0:64]
                    a_t = ewp.tile([HID, PX], F32, tag="a")
                    nc.scalar.activation(
                        a_t[:], p_t[0:HID, :], ACTF.Sigmoid,
                        bias=bias_t[0:HID, 0:1], scale=-1.0,
                    )
                    # [z; sig_h] = sigmoid(psum + [b_g; b_h]) in one op
                    sh_t = ewp.tile([128, PX], BF16, tag="sh")
                    nc.scalar.activation(
                        sh_t[:], p_t[:], ACTF.Sigmoid, bias=bias_t[:, 1:2]
                    )
                    # g = max(hidden + b_h + 0.5, sig_h)   [partitions 64:128]
                    g_t = ewp.tile([128, PX], BF16, tag="g")
                    nc.vector.scalar_tensor_tensor(
                        g_t[HID:, :], p_t[HID:, :], bias_t[HID:, 2:3], sh_t[HID:, :],
                        op0=AL.add, op1=AL.max,
                    )
                    # shift g down to the gate lanes
                    gl_t = ewp.tile([HID, PX], BF16, tag="gl")
                    nc.sync.dma_start(gl_t[:], g_t[HID:, :])
                    # z = 1 - a: alternate engines to balance ACT vs DVE load
                    # bv = z * g   (Pool engine, frees DVE for the scan)
                    bv_t = ewp.tile([HID, PX], F32, tag="bv")
                    nc.gpsimd.tensor_tensor(bv_t[:], sh_t[0:HID, :], gl_t[:], op=AL.mult)
                    # h = a*h_prev + bv
                    tmp_t = ewp.tile([HID, PX], F32, tag="tmp")
                    nc.vector.tensor_tensor(tmp_t[:], a_t[:], hprev[:], op=AL.mult)
                    h_t = outp.tile([HID, PX], F32, tag="h")
                    nc.vector.tensor_tensor(h_t[:], tmp_t[:], bv_t[:], op=AL.add)
                    nc.gpsimd.dma_start(out_d[s, :, :], h_t[:])
                    hprev = h_t



    return nc


def kernel(x, h0, W, b):
    x = np.ascontiguousarray(x, dtype=np.float32)
    import ml_dtypes
    h0 = np.ascontiguousarray(h0, dtype=np.float32)
    W = np.ascontiguousarray(W, dtype=np.float32)
    b = np.ascontiguousarray(b, dtype=np.float32)

    if "nc" not in _CACHE:
        _CACHE["nc"] = _build_nc()
    nc = _CACHE["nc"]

    # host-side prep shared across cores
    wt = W.transpose(1, 2, 3, 0).reshape(C_IN, 9, 2 * HID)  # (ic, ky*3+kx, oc)
    wtp = np.ascontiguousarray(
        np.concatenate([wt[:, [0, 3, 6], :], wt[:, [1, 4, 7], :]], axis=0)
    ).astype(ml_dtypes.bfloat16)  # (128, 3, 128)
    wts = np.ascontiguousarray(wt[:, [2, 5, 8], :]).astype(ml_dtypes.bfloat16)
    biases = np.zeros((128, 4), dtype=np.float32)
    biases[0:HID, 0] = -b[0:HID]
    biases[0:HID, 1] = b[0:HID]
    biases[HID:, 1] = b[HID:]
    biases[HID:, 2] = b[HID:] + 0.5

    in_maps = []
    for core in range(N_CORES):
        bi, hh = core // 2, core % 2
        xpad = np.zeros((S, C_IN, 18, 34), dtype=ml_dtypes.bfloat16)
        r0 = hh * HH - 1  # global row of xpad row 0
        lo, hi = max(r0, 0), min(r0 + 18, H)
        xpad[:, :, lo - r0 : hi - r0, 1:33] = x[bi, :, :, lo:hi, :]
        h0r = np.ascontiguousarray(
            h0[bi, 0, :, hh * HH : (hh + 1) * HH, :].reshape(HID, PX)
        )
        in_maps.append(
            {"xpad": xpad, "wtp": wtp, "wts": wts, "h0r": h0r, "biases": biases}
        )

    res = _bu.run_bass_kernel_spmd(nc, in_maps, core_ids=list(range(N_CORES)))

    out = np.empty((B, S, HID, H, WD), dtype=np.float32)
    for core in range(N_CORES):
        bi, hh = core // 2, core % 2
        core_out = res.results[core]["out"].reshape(S, HID, HH, WD)
        out[bi, :, :, hh * HH : (hh + 1) * HH, :] = core_out
    h_next = out[:, -1:].copy()
    return out, h_next


# revision 29
# speedup vs baseline: 1.2373x; 1.0096x over previous
import json
import sys

sys.path.insert(0, "/opt/trn_rl_repo")

import numpy as np

import concourse.bass_utils as _bu
import concourse.bass2jax as _b2j
import concourse.bass as bass
import concourse.mybir as mybir
from concourse import tile

# ---------------------------------------------------------------------------
# The walrus build in this container only supports ONE sync-wait per
# instruction; current Tile emits multi-wait instructions. Split the extra
# waits into single-wait NoOps on the same engine (engines execute their
# stream in order, so semantics are identical).
_orig_compile_bir = _bu.compile_bir_kernel


def _split_multiwaits(bir_bytes):
    d = json.loads(bir_bytes)
    n = 0
    for fn in d["functions"]:
        for blk in fn["blocks"]:
            out = []
            for ins in blk["instructions"]:
                si = ins.get("sync_info")
                waits = (si or {}).get("on_wait") or []
                if len(waits) > 1:
                    for w in waits[:-1]:
                        n += 1
                        out.append(
                            {
                                "name": f"WSPL{n}-{ins['name']}",
                                "opcode": "NoOp",
                                "engine": ins["engine"],
                                "debug": ins.get("debug", 0),
                                "ins": [],
                                "outs": [],
                                "sync_info": {"on_wait": [w]},
                            }
                        )
                    si["on_wait"] = [waits[-1]]
                out.append(ins)
            blk["instructions"] = out
    return json.dumps(d).encode()


def _patched_compile_bir(bir_json, tmpdir, neff_name="file.neff"):
    return _orig_compile_bir(_split_multiwaits(bir_json), tmpdir, neff_name)


if getattr(_bu.compile_bir_kernel, "__name__", "") != "_patched_compile_bir":
    _bu.compile_bir_kernel = _patched_compile_bir
    _b2j.compile_bir_kernel = _patched_compile_bir
# ---------------------------------------------------------------------------

# nn_MinConv2dGRUCell: x (4,32,64,32,32), h0 (4,1,64,32,32), W (128,64,3,3),
# b (128,). out = (4,32,64,32,32), h_next = out[:, -1:].
# Sharding: 8 cores = B(4) x H-half(2). Per core: conv over its 16 rows
# (+1 halo row each side), then the minGRU scan h_s = a_s*h_{s-1} + bv_s with
# a = sigmoid(-gate-b_g), bv = (1-a)*g, g = max(hidden+b_h+0.5, sigmoid(hidden+b_h)).
B, S, C_IN, H, WD = 4, 32, 64, 32, 32
HID = 64
N_CORES = 8
HH = H // 2  # 16 rows per core
PX = HH * WD  # 512 pixels per core
HPX = PX // 2  # 256 = free size after (128,256) repack
BLK = 8  # timesteps per tensor_tensor_scan instruction
NBLK = S // BLK
SEG = BLK + 1  # +1 reset column per pixel-tuple

F32 = mybir.dt.float32
F32R = mybir.dt.float32r
BF16 = mybir.dt.bfloat16

_CACHE = {}


def _build_nc():
    nc = bass.Bass(trn_type="TRN2")

    xpad_d = nc.dram_tensor("xpad", (S, C_IN, 18, 34), BF16, kind="ExternalInput")
    # paired taps (ky,0)+(ky,1) stacked on K; leftover taps (ky,2) separate
    wtp_d = nc.dram_tensor("wtp", (2 * C_IN, 3, 2 * HID), BF16, kind="ExternalInput")
    wts_d = nc.dram_tensor("wts", (C_IN, 3, 2 * HID), BF16, kind="ExternalInput")
    h0_d = nc.dram_tensor("h0r", (HID, PX), F32, kind="ExternalInput")
    # bias columns: 0 = -b_gate (rows 0:64), 1 = b_hid, 2 = b_hid+0.5
    # (rows 64:128), 3 = +b_gate (rows 0:64)
    bias_d = nc.dram_tensor("biases", (128, 4), F32, kind="ExternalInput")
    # raw scan-layout output: [block, channel, px*SEG] — host strips reset
    # columns and reorders; keeps the device-side DMA fully contiguous.
    out_d = nc.dram_tensor("out", (S, HID, PX), F32, kind="ExternalOutput")

    AL = mybir.AluOpType
    ACTF = mybir.ActivationFunctionType

    with tile.TileContext(nc) as tc:
        with (
            tc.tile_pool(name="const", bufs=1) as constp,
            tc.tile_pool(name="xin", bufs=4) as xinp,
            tc.tile_pool(name="psum", bufs=8, space="PSUM") as psump,
            tc.tile_pool(name="ew", bufs=4) as ewp,
            tc.tile_pool(name="scan", bufs=2) as scanp,
            tc.tile_pool(name="outb", bufs=3) as outp,
        ):
            wtp_t = constp.tile([2 * C_IN, 3 * 2 * HID], BF16)
            nc.sync.dma_start(wtp_t[:], wtp_d[:, :, :])
            wts_t = constp.tile([C_IN, 3 * 2 * HID], BF16)
            nc.sync.dma_start(wts_t[:], wts_d[:, :, :])
            bias_t = constp.tile([128, 4], F32)
            nc.sync.dma_start(bias_t[:], bias_d[:, :])
            h0_t = constp.tile([HID, PX], F32)
            nc.sync.dma_start(h0_t[:], h0_d[:, :])

            hprev = h0_t
            for bs in range(NBLK):
                for j in range(BLK):
                    s = bs * BLK + j
                    # x tile: partitions 0:64 = xpad, 64:128 = xpad shifted
                    # left by one column (tap kx+1 when read at kx)
                    x_t = xinp.tile([2 * C_IN, 18 * 34], BF16)
                    x3 = x_t[:].rearrange("p (r c) -> p r c", r=18)
                    xflat = xpad_d[s, :, :, :].rearrange("c r w -> c (r w)")
                    nc.sync.dma_start(x3[0:C_IN, :, :], xpad_d[s, :, :, :])
                    # shifted copy: bottom[f] = xpad[f+1]; the row-boundary
                    # bleed lands in pad col 33, which no tap reads
                    nc.sync.dma_start(x_t[C_IN:, 0:611], xflat[:, 1:612])

                    p_t = psump.tile([128, PX], F32)
                    for g in range(3):
                        # taps (g,0)+(g,1) in one K=128 matmul
                        nc.tensor.matmul(
                            p_t[:],
                            wtp_t[:, g * 128 : (g + 1) * 128],
                            x3[:, g : g + HH, 0:WD],
                            start=(g == 0),
                            stop=False,
                        )
                    for g in range(3):
                        # leftover tap (g,2), K=64
                        nc.tensor.matmul(
                            p_t[:],
                            wts_t[:, g * 128 : (g + 1) * 128],
                            x3[0:C_IN, g : g + HH, 2 : 2 + WD],
                            start=False,
                            stop=(g == 2),
                        )

                    # a = sigmoid(-gate - b_g)   [partitions 0:64]
                    a_t = ewp.tile([HID, PX], F32, tag="a")
                    nc.scalar.activation(
                        a_t[:], p_t[0:HID, :], ACTF.Sigmoid,
                        bias=bias_t[0:HID, 0:1], scale=-1.0,
                    )
                    # [z; sig_h] = sigmoid(psum + [b_g; b_h]) in one op
                    sh_t = ewp.tile([128, PX], BF16, tag="sh")
                    nc.scalar.activation(
                        sh_t[:], p_t[:], ACTF.Sigmoid, bias=bias_t[:, 1:2]
                    )
                    # g = max(hidden + b_h + 0.5, sig_h)   [partitions 64:128]
                    g_t = ewp.tile([128, PX], BF16, tag="g")
                    nc.vector.scalar_tensor_tensor(
                        g_t[HID:, :], p_t[HID:, :], bias_t[HID:, 2:3], sh_t[HID:, :],
                        op0=AL.add, op1=AL.max,
                    )
                    # shift g down to the gate lanes
                    gl_t = ewp.tile([HID, PX], BF16, tag="gl")
                    nc.scalar.dma_start(gl_t[:], g_t[HID:, :])
                    # z = 1 - a: alternate engines to balance ACT vs DVE load
                    # bv = z * g   (Pool engine, frees DVE for the scan)
                    bv_t = ewp.tile([HID, PX], F32, tag="bv")
                    nc.gpsimd.tensor_tensor(bv_t[:], sh_t[0:HID, :], gl_t[:], op=AL.mult)
                    # h = a*h_prev + bv
                    tmp_t = ewp.tile([HID, PX], F32, tag="tmp")
                    nc.vector.tensor_tensor(tmp_t[:], a_t[:], hprev[:], op=AL.mult)
                    h_t = outp.tile([HID, PX], F32, tag="h")
                    nc.vector.tensor_tensor(h_t[:], tmp_t[:], bv_t[:], op=AL.add)
                    nc.gpsimd.dma_start(out_d[s, :, :], h_t[:])
                    hprev = h_t



    return nc


def kernel(x, h0, W, b):
    x = np.ascontiguousarray(x, dtype=np.float32)
    import ml_dtypes
    h0 = np.ascontiguousarray(h0, dtype=np.float32)
    W = np.ascontiguousarray(W, dtype=np.float32)
    b = np.ascontiguousarray(b, dtype=np.float32)

    if "nc" not in _CACHE:
        _CACHE["nc"] = _build_nc()
    nc = _CACHE["nc"]

    # host-side prep shared across cores
    wt = W.transpose(1, 2, 3, 0).reshape(C_IN, 9, 2 * HID)  # (ic, ky*3+kx, oc)
    wtp = np.ascontiguousarray(
        np.concatenate([wt[:, [0, 3, 6], :], wt[:, [1, 4, 7], :]], axis=0)
    ).astype(ml_dtypes.bfloat16)  # (128, 3, 128)
    wts = np.ascontiguousarray(wt[:, [2, 5, 8], :]).astype(ml_dtypes.bfloat16)
    biases = np.zeros((128, 4), dtype=np.float32)
    biases[0:HID, 0] = -b[0:HID]
    biases[0:HID, 1] = b[0:HID]
    biases[HID:, 1] = b[HID:]
    biases[HID:, 2] = b[HID:] + 0.5

    in_maps = []
    for core in range(N_CORES):
        bi, hh = core // 2, core % 2
        xpad = np.zeros((S, C_IN, 18, 34), dtype=ml_dtypes.bfloat16)
        r0 = hh * HH - 1  # global row of xpad row 0
        lo, hi = max(r0, 0), min(r0 + 18, H)
        xpad[:, :, lo - r0 : hi - r0, 1:33] = x[bi, :, :, lo:hi, :]
        h0r = np.ascontiguousarray(
            h0[bi, 0, :, hh * HH : (hh + 1) * HH, :].reshape(HID, PX)
        )
        in_maps.append(
            {"xpad": xpad, "wtp": wtp, "wts": wts, "h0r": h0r, "biases": biases}
        )

    res = _bu.run_bass_kernel_spmd(nc, in_maps, core_ids=list(range(N_CORES)))

    out = np.empty((B, S, HID, H, WD), dtype=np.float32)
    for core in range(N_CORES):
        bi, hh = core // 2, core % 2
        core_out = res.results[core]["out"].reshape(S, HID, HH, WD)
        out[bi, :, :, hh * HH : (hh + 1) * HH, :] = core_out
    h_next = out[:, -1:].copy()
    return out, h_next


# revision 31
# speedup vs baseline: 1.4990x; 1.2116x over previous
import json
import sys

sys.path.insert(0, "/opt/trn_rl_repo")

import numpy as np

import concourse.bass_utils as _bu
import concourse.bass2jax as _b2j
import concourse.bass as bass
import concourse.mybir as mybir
from concourse import tile

# ---------------------------------------------------------------------------
# The walrus build in this container only supports ONE sync-wait per
# instruction; current Tile emits multi-wait instructions. Split the extra
# waits into single-wait NoOps on the same engine (engines execute their
# stream in order, so semantics are identical).
_orig_compile_bir = _bu.compile_bir_kernel


def _split_multiwaits(bir_bytes):
    d = json.loads(bir_bytes)
    n = 0
    for fn in d["functions"]:
        for blk in fn["blocks"]:
            out = []
            for ins in blk["instructions"]:
                si = ins.get("sync_info")
                waits = (si or {}).get("on_wait") or []
                if len(waits) > 1:
                    for w in waits[:-1]:
                        n += 1
                        out.append(
                            {
                                "name": f"WSPL{n}-{ins['name']}",
                                "opcode": "NoOp",
                                "engine": ins["engine"],
                                "debug": ins.get("debug", 0),
                                "ins": [],
                                "outs": [],
                                "sync_info": {"on_wait": [w]},
                            }
                        )
                    si["on_wait"] = [waits[-1]]
                out.append(ins)
            blk["instructions"] = out
    return json.dumps(d).encode()


def _patched_compile_bir(bir_json, tmpdir, neff_name="file.neff"):
    return _orig_compile_bir(_split_multiwaits(bir_json), tmpdir, neff_name)


if getattr(_bu.compile_bir_kernel, "__name__", "") != "_patched_compile_bir":
    _bu.compile_bir_kernel = _patched_compile_bir
    _b2j.compile_bir_kernel = _patched_compile_bir
# ---------------------------------------------------------------------------

# nn_MinConv2dGRUCell: x (4,32,64,32,32), h0 (4,1,64,32,32), W (128,64,3,3),
# b (128,). out = (4,32,64,32,32), h_next = out[:, -1:].
# Sharding: 8 cores = B(4) x H-half(2). Per core: conv over its 16 rows
# (+1 halo row each side), then the minGRU scan h_s = a_s*h_{s-1} + bv_s with
# a = sigmoid(-gate-b_g), bv = (1-a)*g, g = max(hidden+b_h+0.5, sigmoid(hidden+b_h)).
B, S, C_IN, H, WD = 4, 32, 64, 32, 32
HID = 64
N_CORES = 8
HH = H // 2  # 16 rows per core
PX = HH * WD  # 512 pixels per core
HPX = PX // 2  # 256 = free size after (128,256) repack
BLK = 8  # timesteps per tensor_tensor_scan instruction
NBLK = S // BLK
SEG = BLK + 1  # +1 reset column per pixel-tuple

F32 = mybir.dt.float32
F32R = mybir.dt.float32r
BF16 = mybir.dt.bfloat16

_CACHE = {}


def _build_nc():
    nc = bass.Bass(trn_type="TRN2")

    xpad_d = nc.dram_tensor("xpad", (S, C_IN, 18, 34), BF16, kind="ExternalInput")
    # paired taps (ky,0)+(ky,1) stacked on K; leftover taps (ky,2) separate
    wtp_d = nc.dram_tensor("wtp", (2 * C_IN, 3, 2 * HID), BF16, kind="ExternalInput")
    wts_d = nc.dram_tensor("wts", (C_IN, 3, 2 * HID), BF16, kind="ExternalInput")
    h0_d = nc.dram_tensor("h0r", (128, HPX), F32, kind="ExternalInput")
    # bias columns: 0 = -b_gate (rows 0:64), 1 = b_hid, 2 = b_hid+0.5
    # (rows 64:128), 3 = +b_gate (rows 0:64)
    bias_d = nc.dram_tensor("biases", (128, 4), F32, kind="ExternalInput")
    # raw scan-layout output: [block, channel, px*SEG] — host strips reset
    # columns and reorders; keeps the device-side DMA fully contiguous.
    out_d = nc.dram_tensor("out", (S, 128, HPX), F32, kind="ExternalOutput")

    AL = mybir.AluOpType
    ACTF = mybir.ActivationFunctionType

    with tile.TileContext(nc) as tc:
        with (
            tc.tile_pool(name="const", bufs=1) as constp,
            tc.tile_pool(name="xin", bufs=4) as xinp,
            tc.tile_pool(name="psum", bufs=8, space="PSUM") as psump,
            tc.tile_pool(name="ew", bufs=4) as ewp,
            tc.tile_pool(name="scan", bufs=2) as scanp,
            tc.tile_pool(name="outb", bufs=3) as outp,
        ):
            wtp_t = constp.tile([2 * C_IN, 3 * 2 * HID], BF16)
            nc.sync.dma_start(wtp_t[:], wtp_d[:, :, :])
            wts_t = constp.tile([C_IN, 3 * 2 * HID], BF16)
            nc.sync.dma_start(wts_t[:], wts_d[:, :, :])
            bias_t = constp.tile([128, 4], F32)
            nc.sync.dma_start(bias_t[:], bias_d[:, :])
            h0_t = constp.tile([128, HPX], F32)
            nc.sync.dma_start(h0_t[:], h0_d[:, :])

            hprev = h0_t
            for bs in range(NBLK):
                for j in range(BLK):
                    s = bs * BLK + j
                    # x tile: partitions 0:64 = xpad, 64:128 = xpad shifted
                    # left by one column (tap kx+1 when read at kx)
                    x_t = xinp.tile([2 * C_IN, 18 * 34], BF16)
                    x3 = x_t[:].rearrange("p (r c) -> p r c", r=18)
                    xflat = xpad_d[s, :, :, :].rearrange("c r w -> c (r w)")
                    nc.sync.dma_start(x3[0:C_IN, :, :], xpad_d[s, :, :, :])
                    # shifted copy: bottom[f] = xpad[f+1]; the row-boundary
                    # bleed lands in pad col 33, which no tap reads
                    nc.sync.dma_start(x_t[C_IN:, 0:611], xflat[:, 1:612])

                    p_t = psump.tile([128, PX], F32)
                    for g in range(3):
                        # taps (g,0)+(g,1) in one K=128 matmul
                        nc.tensor.matmul(
                            p_t[:],
                            wtp_t[:, g * 128 : (g + 1) * 128],
                            x3[:, g : g + HH, 0:WD],
                            start=(g == 0),
                            stop=False,
                        )
                    for g in range(3):
                        # leftover tap (g,2), K=64
                        nc.tensor.matmul(
                            p_t[:],
                            wts_t[:, g * 128 : (g + 1) * 128],
                            x3[0:C_IN, g : g + HH, 2 : 2 + WD],
                            start=False,
                            stop=(g == 2),
                        )

                    # [z; sig_h] = sigmoid(psum + [b_g; b_h]) in one op
                    sh_t = ewp.tile([128, PX], F32, tag="sh")
                    nc.scalar.activation(
                        sh_t[:], p_t[:], ACTF.Sigmoid, bias=bias_t[:, 1:2]
                    )
                    # g = max(hidden + b_h + 0.5, sig_h)   [partitions 64:128]
                    g_t = ewp.tile([128, PX], BF16, tag="g")
                    nc.vector.scalar_tensor_tensor(
                        g_t[HID:, :], p_t[HID:, :], bias_t[HID:, 2:3], sh_t[HID:, :],
                        op0=AL.add, op1=AL.max,
                    )
                    # repack z and g to 128-partition (q = 2c + px_half) layout
                    z2_t = ewp.tile([128, HPX], F32, tag="z2")
                    nc.scalar.dma_start(
                        z2_t[:], sh_t[0:HID, :].rearrange("c (h i) -> c h i", h=2)
                    )
                    g2_t = ewp.tile([128, HPX], BF16, tag="g2")
                    nc.gpsimd.dma_start(
                        g2_t[:], g_t[HID:, :].rearrange("c (h i) -> c h i", h=2)
                    )
                    # bv = z * g  (Pool)
                    bv_t = ewp.tile([128, HPX], F32, tag="bv")
                    nc.gpsimd.tensor_tensor(bv_t[:], z2_t[:], g2_t[:], op=AL.mult)
                    # h = (1-z)*h_prev + bv  ==  bv - (z-1)*h_prev
                    tmp_t = ewp.tile([128, HPX], F32, tag="tmp")
                    nc.vector.scalar_tensor_tensor(
                        tmp_t[:], z2_t[:], 1.0, hprev[:], op0=AL.subtract, op1=AL.mult
                    )
                    h_t = outp.tile([128, HPX], F32, tag="h")
                    nc.vector.tensor_tensor(h_t[:], bv_t[:], tmp_t[:], op=AL.subtract)
                    nc.sync.dma_start(out_d[s, :, :], h_t[:])
                    hprev = h_t



    return nc


def kernel(x, h0, W, b):
    x = np.ascontiguousarray(x, dtype=np.float32)
    import ml_dtypes
    h0 = np.ascontiguousarray(h0, dtype=np.float32)
    W = np.ascontiguousarray(W, dtype=np.float32)
    b = np.ascontiguousarray(b, dtype=np.float32)

    if "nc" not in _CACHE:
        _CACHE["nc"] = _build_nc()
    nc = _CACHE["nc"]

    # host-side prep shared across cores
    wt = W.transpose(1, 2, 3, 0).reshape(C_IN, 9, 2 * HID)  # (ic, ky*3+kx, oc)
    wtp = np.ascontiguousarray(
        np.concatenate([wt[:, [0, 3, 6], :], wt[:, [1, 4, 7], :]], axis=0)
    ).astype(ml_dtypes.bfloat16)  # (128, 3, 128)
    wts = np.ascontiguousarray(wt[:, [2, 5, 8], :]).astype(ml_dtypes.bfloat16)
    biases = np.zeros((128, 4), dtype=np.float32)
    biases[0:HID, 0] = -b[0:HID]
    biases[0:HID, 1] = b[0:HID]
    biases[HID:, 1] = b[HID:]
    biases[HID:, 2] = b[HID:] + 0.5

    in_maps = []
    for core in range(N_CORES):
        bi, hh = core // 2, core % 2
        xpad = np.zeros((S, C_IN, 18, 34), dtype=ml_dtypes.bfloat16)
        r0 = hh * HH - 1  # global row of xpad row 0
        lo, hi = max(r0, 0), min(r0 + 18, H)
        xpad[:, :, lo - r0 : hi - r0, 1:33] = x[bi, :, :, lo:hi, :]
        h0r = np.ascontiguousarray(
            h0[bi, 0, :, hh * HH : (hh + 1) * HH, :].reshape(128, HPX)
        )
        in_maps.append(
            {"xpad": xpad, "wtp": wtp, "wts": wts, "h0r": h0r, "biases": biases}
        )

    res = _bu.run_bass_kernel_spmd(nc, in_maps, core_ids=list(range(N_CORES)))

    out = np.empty((B, S, HID, H, WD), dtype=np.float32)
    for core in range(N_CORES):
        bi, hh = core // 2, core % 2
        core_out = res.results[core]["out"].reshape(S, HID, HH, WD)
        out[bi, :, :, hh * HH : (hh + 1) * HH, :] = core_out
    h_next = out[:, -1:].copy()
    return out, h_next


# revision 35
# speedup vs baseline: 1.5044x; 1.0036x over previous
import json
import sys

sys.path.insert(0, "/opt/trn_rl_repo")

import ml_dtypes
import numpy as np

import concourse.bass_utils as _bu
import concourse.bass2jax as _b2j
import concourse.bass as bass
import concourse.mybir as mybir
from concourse import tile

# ---------------------------------------------------------------------------
# The walrus build in this container only supports ONE sync-wait per
# instruction; current Tile emits multi-wait instructions. Split the extra
# waits into single-wait NoOps on the same engine (engines execute their
# stream in order, so semantics are identical).
_orig_compile_bir = _bu.compile_bir_kernel


def _split_multiwaits(bir_bytes):
    d = json.loads(bir_bytes)
    n = 0
    for fn in d["functions"]:
        for blk in fn["blocks"]:
            out = []
            for ins in blk["instructions"]:
                si = ins.get("sync_info")
                waits = (si or {}).get("on_wait") or []
                if len(waits) > 1:
                    for w in waits[:-1]:
                        n += 1
                        out.append(
                            {
                                "name": f"WSPL{n}-{ins['name']}",
                                "opcode": "NoOp",
                                "engine": ins["engine"],
                                "debug": ins.get("debug", 0),
                                "ins": [],
                                "outs": [],
                                "sync_info": {"on_wait": [w]},
                            }
                        )
                    si["on_wait"] = [waits[-1]]
                out.append(ins)
            blk["instructions"] = out
    return json.dumps(d).encode()


def _patched_compile_bir(bir_json, tmpdir, neff_name="file.neff"):
    return _orig_compile_bir(_split_multiwaits(bir_json), tmpdir, neff_name)


if getattr(_bu.compile_bir_kernel, "__name__", "") != "_patched_compile_bir":
    _bu.compile_bir_kernel = _patched_compile_bir
    _b2j.compile_bir_kernel = _patched_compile_bir
# ---------------------------------------------------------------------------

# nn_MinConv2dGRUCell: x (4,32,64,32,32), h0 (4,1,64,32,32), W (128,64,3,3),
# b (128,). out = (4,32,64,32,32), h_next = out[:, -1:].
# Sharding: 8 cores = B(4) x H-half(2). Per core: 3x3 conv over its 16 rows
# (+1 halo row each side) for all 32 timesteps, then the minGRU recurrence
# h_s = (1-z_s)*h_{s-1} + z_s*g_s with z = sigmoid(gate+b_g) and
# g = max(hidden+b_h+0.5, sigmoid(hidden+b_h)).
B, S, C_IN, H, WD = 4, 32, 64, 32, 32
HID = 64
N_CORES = 8
HH = H // 2  # 16 rows per core
PX = HH * WD  # 512 pixels per core
HPX = PX // 2  # 256 = free size in the packed (128, 256) layout

F32 = mybir.dt.float32
BF16 = mybir.dt.bfloat16

_CACHE = {}


def _build_nc():
    nc = bass.Bass(trn_type="TRN2")

    xpad_d = nc.dram_tensor("xpad", (S, C_IN, 18, 34), BF16, kind="ExternalInput")
    # paired taps (ky,0)+(ky,1) stacked on K; leftovers (0,2),(2,2) and (1,2)
    wtp_d = nc.dram_tensor("wtp", (2 * C_IN, 3, 2 * HID), BF16, kind="ExternalInput")
    wts_d = nc.dram_tensor("wts", (C_IN, 3, 2 * HID), BF16, kind="ExternalInput")
    h0_d = nc.dram_tensor("h0r", (128, HPX), F32, kind="ExternalInput")
    # bias columns: 1 = [b_gate; b_hid], 2 = b_hid+0.5 (rows 64:128)
    bias_d = nc.dram_tensor("biases", (128, 4), F32, kind="ExternalInput")
    # packed per-step output (q = 2c + px_half); host reorders
    out_d = nc.dram_tensor("out", (S, 128, HPX), F32, kind="ExternalOutput")

    AL = mybir.AluOpType
    ACTF = mybir.ActivationFunctionType

    with tile.TileContext(nc) as tc:
        with (
            tc.tile_pool(name="const", bufs=1) as constp,
            tc.tile_pool(name="xin", bufs=4) as xinp,
            tc.tile_pool(name="psum", bufs=8, space="PSUM") as psump,
            tc.tile_pool(name="ew", bufs=4) as ewp,
            tc.tile_pool(name="outb", bufs=3) as outp,
        ):
            wtp_t = constp.tile([2 * C_IN, 3 * 2 * HID], BF16)
            nc.sync.dma_start(wtp_t[:], wtp_d[:, :, :])
            wts_t = constp.tile([C_IN, 3 * 2 * HID], BF16)
            nc.sync.dma_start(wts_t[:], wts_d[:, :, :])
            bias_t = constp.tile([128, 4], F32)
            nc.sync.dma_start(bias_t[:], bias_d[:, :])
            h0_t = constp.tile([128, HPX], F32)
            nc.sync.dma_start(h0_t[:], h0_d[:, :])

            hprev = h0_t
            for sp in range(S // 2):
                # stage A: x DMAs for the timestep pair
                pair = []
                for s in (2 * sp, 2 * sp + 1):
                    # x tile: partitions 0:64 = xpad, 64:128 = xpad shifted
                    # left one element (reads tap kx+1 when addressed at kx)
                    x_t = xinp.tile([2 * C_IN, 18 * 34], BF16, tag="x")
                    x3 = x_t[:].rearrange("p (r c) -> p r c", r=18)
                    xflat = xpad_d[s, :, :, :].rearrange("c r w -> c (r w)")
                    nc.sync.dma_start(x3[0:C_IN, :, :], xpad_d[s, :, :, :])
                    # row-boundary bleed lands in pad col 33 (never read)
                    nc.sync.dma_start(x_t[C_IN:, 0:611], xflat[:, 1:612])
                    pair.append((s, x3))

                # stage B: all matmuls for the pair, back-to-back on PE
                ppair = []
                for s, x3 in pair:
                    p_t = psump.tile([128, PX], F32)
                    for g in range(3):
                        # taps (g,0)+(g,1) in one K=128 matmul
                        nc.tensor.matmul(
                            p_t[:],
                            wtp_t[:, g * 128 : (g + 1) * 128],
                            x3[:, g : g + HH, 0:WD],
                            start=(g == 0),
                            stop=False,
                        )
                    for g in range(3):
                        # leftover tap (g,2), K=64
                        nc.tensor.matmul(
                            p_t[:],
                            wts_t[:, g * 128 : (g + 1) * 128],
                            x3[0:C_IN, g : g + HH, 2 : 2 + WD],
                            start=False,
                            stop=(g == 2),
                        )
                    ppair.append((s, p_t))

                # stage C: elementwise + recurrence per timestep
                for s, p_t in ppair:
                    # [z; sig_h] = sigmoid(psum + [b_g; b_h]) in one op
                    sh_t = ewp.tile([128, PX], F32, tag="sh")
                    nc.scalar.activation(
                        sh_t[:], p_t[:], ACTF.Sigmoid, bias=bias_t[:, 1:2]
                    )
                    # g = max(hidden + b_h + 0.5, sig_h)   [partitions 64:128]
                    g_t = ewp.tile([128, PX], BF16, tag="g")
                    nc.vector.scalar_tensor_tensor(
                        g_t[HID:, :], p_t[HID:, :], bias_t[HID:, 2:3],
                        sh_t[HID:, :], op0=AL.add, op1=AL.max,
                    )
                    # repack z and g to the 128-partition (q = 2c+half) layout
                    z2_t = ewp.tile([128, HPX], F32, tag="z2")
                    nc.scalar.dma_start(
                        z2_t[:], sh_t[0:HID, :].rearrange("c (h i) -> c h i", h=2)
                    )
                    g2_t = ewp.tile([128, HPX], BF16, tag="g2")
                    nc.gpsimd.dma_start(
                        g2_t[:], g_t[HID:, :].rearrange("c (h i) -> c h i", h=2)
                    )
                    # bv = z * g  (Pool)
                    bv_t = ewp.tile([128, HPX], F32, tag="bv")
                    nc.gpsimd.tensor_tensor(bv_t[:], z2_t[:], g2_t[:], op=AL.mult)
                    # h = (1-z)*h_prev + bv  ==  bv - (z-1)*h_prev
                    tmp_t = ewp.tile([128, HPX], F32, tag="tmp")
                    nc.vector.scalar_tensor_tensor(
                        tmp_t[:], z2_t[:], 1.0, hprev[:],
                        op0=AL.subtract, op1=AL.mult,
                    )
                    h_t = outp.tile([128, HPX], F32, tag="h")
                    nc.vector.tensor_tensor(h_t[:], bv_t[:], tmp_t[:], op=AL.subtract)
                    nc.sync.dma_start(out_d[s, :, :], h_t[:])
                    hprev = h_t

    return nc


def kernel(x, h0, W, b):
    x = np.ascontiguousarray(x, dtype=np.float32)
    h0 = np.ascontiguousarray(h0, dtype=np.float32)
    W = np.ascontiguousarray(W, dtype=np.float32)
    b = np.ascontiguousarray(b, dtype=np.float32)

    if "nc" not in _CACHE:
        _CACHE["nc"] = _build_nc()
    nc = _CACHE["nc"]

    # host-side prep shared across cores
    wt = W.transpose(1, 2, 3, 0).reshape(C_IN, 9, 2 * HID)  # (ic, ky*3+kx, oc)
    wtp = np.ascontiguousarray(
        np.concatenate([wt[:, [0, 3, 6], :], wt[:, [1, 4, 7], :]], axis=0)
    ).astype(ml_dtypes.bfloat16)  # (128, 3, 128): [taps (ky,0); taps (ky,1)]
    wts = np.ascontiguousarray(wt[:, [2, 5, 8], :]).astype(ml_dtypes.bfloat16)
    biases = np.zeros((128, 4), dtype=np.float32)
    biases[0:HID, 1] = b[0:HID]
    biases[HID:, 1] = b[HID:]
    biases[HID:, 2] = b[HID:] + 0.5

    in_maps = []
    for core in range(N_CORES):
        bi, hh = core // 2, core % 2
        xpad = np.zeros((S, C_IN, 18, 34), dtype=ml_dtypes.bfloat16)
        r0 = hh * HH - 1  # global row of xpad row 0
        lo, hi = max(r0, 0), min(r0 + 18, H)
        xpad[:, :, lo - r0 : hi - r0, 1:33] = x[bi, :, :, lo:hi, :]
        h0r = np.ascontiguousarray(
            h0[bi, 0, :, hh * HH : (hh + 1) * HH, :].reshape(128, HPX)
        )
        in_maps.append(
            {
                "xpad": xpad,
                "wtp": wtp,
                "wts": wts,
                "h0r": h0r,
                "biases": biases,
            }
        )

    res = _bu.run_bass_kernel_spmd(nc, in_maps, core_ids=list(range(N_CORES)))

    out = np.empty((B, S, HID, H, WD), dtype=np.float32)
    for core in range(N_CORES):
        bi, hh = core // 2, core % 2
        core_out = res.results[core]["out"].reshape(S, HID, HH, WD)
        out[bi, :, :, hh * HH : (hh + 1) * HH, :] = core_out
    h_next = out[:, -1:].copy()
    return out, h_next


# revision 40
# speedup vs baseline: 1.6285x; 1.0824x over previous
import json
import sys

sys.path.insert(0, "/opt/trn_rl_repo")

import ml_dtypes
import numpy as np

import concourse.bass_utils as _bu
import concourse.bass2jax as _b2j
import concourse.bass as bass
import concourse.mybir as mybir
from concourse import tile

# ---------------------------------------------------------------------------
# The walrus build in this container only supports ONE sync-wait per
# instruction; current Tile emits multi-wait instructions. Split the extra
# waits into single-wait NoOps on the same engine (engines execute their
# stream in order, so semantics are identical).
_orig_compile_bir = _bu.compile_bir_kernel


def _split_multiwaits(bir_bytes):
    d = json.loads(bir_bytes)
    n = 0
    for fn in d["functions"]:
        for blk in fn["blocks"]:
            out = []
            for ins in blk["instructions"]:
                si = ins.get("sync_info")
                waits = (si or {}).get("on_wait") or []
                if len(waits) > 1:
                    for w in waits[:-1]:
                        n += 1
                        out.append(
                            {
                                "name": f"WSPL{n}-{ins['name']}",
                                "opcode": "NoOp",
                                "engine": ins["engine"],
                                "debug": ins.get("debug", 0),
                                "ins": [],
                                "outs": [],
                                "sync_info": {"on_wait": [w]},
                            }
                        )
                    si["on_wait"] = [waits[-1]]
                out.append(ins)
            blk["instructions"] = out
    return json.dumps(d).encode()


def _patched_compile_bir(bir_json, tmpdir, neff_name="file.neff"):
    return _orig_compile_bir(_split_multiwaits(bir_json), tmpdir, neff_name)


if getattr(_bu.compile_bir_kernel, "__name__", "") != "_patched_compile_bir":
    _bu.compile_bir_kernel = _patched_compile_bir
    _b2j.compile_bir_kernel = _patched_compile_bir

    _orig_run_command = _bu.run_command

    def _patched_run_command(argv, **kw):
        argv = [
            a
            for a in argv
        ]
        return _orig_run_command(argv, **kw)

    _bu.run_command = _patched_run_command
# ---------------------------------------------------------------------------

# nn_MinConv2dGRUCell: x (4,32,64,32,32), h0 (4,1,64,32,32), W (128,64,3,3),
# b (128,). out = (4,32,64,32,32), h_next = out[:, -1:].
# Sharding: 8 cores = B(4) x H-half(2). Per core: 3x3 conv over its 16 rows
# (+1 halo row each side) for all 32 timesteps, then the minGRU recurrence
# h_s = (1-z_s)*h_{s-1} + z_s*g_s with z = sigmoid(gate+b_g) and
# g = max(hidden+b_h+0.5, sigmoid(hidden+b_h)).
B, S, C_IN, H, WD = 4, 32, 64, 32, 32
HID = 64
N_CORES = 8
HH = H // 2  # 16 rows per core
PX = HH * WD  # 512 pixels per core
HPX = PX // 2  # 256 = free size in the packed (128, 256) layout

F32 = mybir.dt.float32
BF16 = mybir.dt.bfloat16

_CACHE = {}


def _build_nc():
    nc = bass.Bass(trn_type="TRN2")

    xpad_d = nc.dram_tensor("xpad", (S, C_IN, 18, 34), BF16, kind="ExternalInput")
    # paired taps (ky,0)+(ky,1) stacked on K; leftovers (0,2),(2,2) and (1,2)
    wtp_d = nc.dram_tensor("wtp", (2 * C_IN, 3, 2 * HID), BF16, kind="ExternalInput")
    wts_d = nc.dram_tensor("wts", (C_IN, 3, 2 * HID), BF16, kind="ExternalInput")
    h0_d = nc.dram_tensor("h0r", (128, HPX), F32, kind="ExternalInput")
    # bias columns: 1 = [b_gate; b_hid], 2 = b_hid+0.5 (rows 64:128)
    bias_d = nc.dram_tensor("biases", (128, 4), F32, kind="ExternalInput")
    # packed per-step output (q = 2c + px_half); host reorders
    out_d = nc.dram_tensor("out", (S, 128, HPX), F32, kind="ExternalOutput")

    AL = mybir.AluOpType
    ACTF = mybir.ActivationFunctionType

    with tile.TileContext(nc) as tc:
        with (
            tc.tile_pool(name="const", bufs=1) as constp,
            tc.tile_pool(name="xin", bufs=4) as xinp,
            tc.tile_pool(name="psum", bufs=8, space="PSUM") as psump,
            tc.tile_pool(name="ew", bufs=4) as ewp,
            tc.tile_pool(name="outb", bufs=3) as outp,
        ):
            wtp_t = constp.tile([2 * C_IN, 3 * 2 * HID], BF16)
            nc.sync.dma_start(wtp_t[:], wtp_d[:, :, :])
            wts_t = constp.tile([C_IN, 3 * 2 * HID], BF16)
            nc.sync.dma_start(wts_t[:], wts_d[:, :, :])
            bias_t = constp.tile([128, 4], F32)
            nc.sync.dma_start(bias_t[:], bias_d[:, :])
            h0_t = constp.tile([128, HPX], F32)
            nc.sync.dma_start(h0_t[:], h0_d[:, :])

            # HAM warm-up: ~7us of dense matmuls on garbage data so the
            # PE clock-gate opens before the real conv stream begins
            warm_t = psump.tile([128, 384], F32, tag="pt")
            for wi in range(16):
                nc.tensor.matmul(
                    warm_t[:],
                    wtp_t[:, 0:128],
                    wtp_t[:, 0:384],
                    start=True,
                    stop=(wi == 15),
                )

            hprev = h0_t
            for sp in range(S // 2):
                # stage A: x DMAs for the timestep pair
                pair = []
                for s in (2 * sp, 2 * sp + 1):
                    # x tile: partitions 0:64 = xpad, 64:128 = xpad shifted
                    # left one element (reads tap kx+1 when addressed at kx)
                    x_t = xinp.tile([2 * C_IN, 18 * 34], BF16, tag="x")
                    x3 = x_t[:].rearrange("p (r c) -> p r c", r=18)
                    xflat = xpad_d[s, :, :, :].rearrange("c r w -> c (r w)")
                    nc.sync.dma_start(x3[0:C_IN, :, :], xpad_d[s, :, :, :])
                    # row-boundary bleed lands in pad col 33 (never read)
                    nc.sync.dma_start(x_t[C_IN:, 0:611], xflat[:, 1:612])
                    pair.append((s, x3))

                # stage B: matmuls tap-outer over the pair (weight reuse)
                ppair = [(s, psump.tile([128, PX], F32, name=f"pt{s}", tag="pt")) for s, _ in pair]
                for g in range(3):
                    for k, (s, x3) in enumerate(pair):
                        # taps (g,0)+(g,1) in one K=128 matmul
                        nc.tensor.matmul(
                            ppair[k][1][:],
                            wtp_t[:, g * 128 : (g + 1) * 128],
                            x3[:, g : g + HH, 0:WD],
                            start=(g == 0),
                            stop=False,
                        )
                for g in range(3):
                    for k, (s, x3) in enumerate(pair):
                        # leftover tap (g,2), K=64
                        nc.tensor.matmul(
                            ppair[k][1][:],
                            wts_t[:, g * 128 : (g + 1) * 128],
                            x3[0:C_IN, g : g + HH, 2 : 2 + WD],
                            start=False,
                            stop=(g == 2),
                        )

                # stage C: elementwise + recurrence per timestep
                for s, p_t in ppair:
                    # [z; sig_h] = sigmoid(psum + [b_g; b_h]) in one op
                    sh_t = ewp.tile([128, PX], F32, tag="sh")
                    nc.scalar.activation(
                        sh_t[:], p_t[:], ACTF.Sigmoid, bias=bias_t[:, 1:2]
                    )
                    # g = max(hidden + b_h + 0.5, sig_h)   [partitions 64:128]
                    g_t = ewp.tile([128, PX], BF16, tag="g")
                    nc.vector.scalar_tensor_tensor(
                        g_t[HID:, :], p_t[HID:, :], bias_t[HID:, 2:3],
                        sh_t[HID:, :], op0=AL.add, op1=AL.max,
                    )
                    # repack z and g to the 128-partition (q = 2c+half) layout
                    z2_t = ewp.tile([128, HPX], F32, tag="z2")
                    nc.scalar.dma_start(
                        z2_t[:], sh_t[0:HID, :].rearrange("c (h i) -> c h i", h=2)
                    )
                    g2_t = ewp.tile([128, HPX], BF16, tag="g2")
                    nc.gpsimd.dma_start(
                        g2_t[:], g_t[HID:, :].rearrange("c (h i) -> c h i", h=2)
                    )
                    # bv = z * g  (Pool)
                    bv_t = ewp.tile([128, HPX], F32, tag="bv")
                    nc.gpsimd.tensor_tensor(bv_t[:], z2_t[:], g2_t[:], op=AL.mult)
                    # h = (1-z)*h_prev + bv  ==  bv - (z-1)*h_prev
                    tmp_t = ewp.tile([128, HPX], F32, tag="tmp")
                    nc.vector.scalar_tensor_tensor(
                        tmp_t[:], z2_t[:], 1.0, hprev[:],
                        op0=AL.subtract, op1=AL.mult,
                    )
                    h_t = outp.tile([128, HPX], F32, tag="h")
                    nc.vector.tensor_tensor(h_t[:], bv_t[:], tmp_t[:], op=AL.subtract)
                    nc.sync.dma_start(out_d[s, :, :], h_t[:])
                    hprev = h_t

    return nc


def kernel(x, h0, W, b):
    x = np.ascontiguousarray(x, dtype=np.float32)
    h0 = np.ascontiguousarray(h0, dtype=np.float32)
    W = np.ascontiguousarray(W, dtype=np.float32)
    b = np.ascontiguousarray(b, dtype=np.float32)

    if "nc" not in _CACHE:
        _CACHE["nc"] = _build_nc()
    nc = _CACHE["nc"]

    # host-side prep shared across cores
    wt = W.transpose(1, 2, 3, 0).reshape(C_IN, 9, 2 * HID)  # (ic, ky*3+kx, oc)
    wtp = np.ascontiguousarray(
        np.concatenate([wt[:, [0, 3, 6], :], wt[:, [1, 4, 7], :]], axis=0)
    ).astype(ml_dtypes.bfloat16)  # (128, 3, 128): [taps (ky,0); taps (ky,1)]
    wts = np.ascontiguousarray(wt[:, [2, 5, 8], :]).astype(ml_dtypes.bfloat16)
    biases = np.zeros((128, 4), dtype=np.float32)
    biases[0:HID, 1] = b[0:HID]
    biases[HID:, 1] = b[HID:]
    biases[HID:, 2] = b[HID:] + 0.5

    in_maps = []
    for core in range(N_CORES):
        bi, hh = core // 2, core % 2
        xpad = np.zeros((S, C_IN, 18, 34), dtype=ml_dtypes.bfloat16)
        r0 = hh * HH - 1  # global row of xpad row 0
        lo, hi = max(r0, 0), min(r0 + 18, H)
        xpad[:, :, lo - r0 : hi - r0, 1:33] = x[bi, :, :, lo:hi, :]
        h0r = np.ascontiguousarray(
            h0[bi, 0, :, hh * HH : (hh + 1) * HH, :].reshape(128, HPX)
        )
        in_maps.append(
            {
                "xpad": xpad,
                "wtp": wtp,
                "wts": wts,
                "h0r": h0r,
                "biases": biases,
            }
        )

    res = _bu.run_bass_kernel_spmd(nc, in_maps, core_ids=list(range(N_CORES)))

    out = np.empty((B, S, HID, H, WD), dtype=np.float32)
    for core in range(N_CORES):
        bi, hh = core // 2, core % 2
        core_out = res.results[core]["out"].reshape(S, HID, HH, WD)
        out[bi, :, :, hh * HH : (hh + 1) * HH, :] = core_out
    h_next = out[:, -1:].copy()
    return out, h_next
